# revision 10
# baseline (speedup 1.0000x reference)
import numpy as np

import concourse.bass as bass
import concourse.bacc as bacc
import concourse.mybir as mybir
from concourse import tile
from concourse.masks import make_identity
from concourse.bass_utils import run_bass_kernel_spmd

B, C, H, W = 4, 256, 224, 224
PH = PW = 7
P = 49
MH = MW = 32
NCORES = 8
RPC = (B * MH) // NCORES  # 16 row-units (b,mh) per core
G = 8                     # patches per group
NG = MW // G              # 4 groups per row-unit

f32 = mybir.dt.float32
ALU = mybir.AluOpType
ACT = mybir.ActivationFunctionType
AX = mybir.AxisListType

_cached_nc = None


def _build_nc():
    nc = bacc.Bacc("TRN2", target_bir_lowering=False, debug=False)
    # patch-major: x[r, c, mw, p] with p = i*PW + j token index in patch
    x_in = nc.dram_tensor("x", [RPC, C, MW, P], f32, kind="ExternalInput")
    b_in = nc.dram_tensor("beta", [1], f32, kind="ExternalInput")
    out_o = nc.dram_tensor("out", [RPC, C, MW, P], f32, kind="ExternalOutput")
    ec_o = nc.dram_tensor("ec", [RPC, C, MW, P], f32, kind="ExternalOutput")
    sc_o = nc.dram_tensor("sc", [RPC, MW, P, P], f32, kind="ExternalOutput")
    cov_o = nc.dram_tensor("cov", [RPC, MW, P, P], f32, kind="ExternalOutput")
    l_o = nc.dram_tensor("l", [RPC, MW, P, P], f32, kind="ExternalOutput")

    with tile.TileContext(nc) as tc:
        with (
            tc.tile_pool(name="const", bufs=1) as constp,
            tc.tile_pool(name="rows", bufs=2) as rows,
            tc.tile_pool(name="grp", bufs=2) as grp,
            tc.tile_pool(name="psS", bufs=2, space=bass.MemorySpace.PSUM) as psS,
            tc.tile_pool(name="psT", bufs=2, space=bass.MemorySpace.PSUM) as psT,
            tc.tile_pool(name="psA", bufs=2, space=bass.MemorySpace.PSUM) as psA,
            tc.tile_pool(name="psM", bufs=2, space=bass.MemorySpace.PSUM) as psM,
        ):
            ident = constp.tile([128, 128], f32)
            make_identity(nc, ident[:])
            beta_sb = constp.tile([128, 1], f32)
            nc.sync.dma_start(beta_sb[:, 0:1],
                              b_in[:].unsqueeze(0).broadcast_to([128, 1]))

            for r in range(RPC):
                xrow = [rows.tile([128, MW, P], f32, name=f"xrow{c}") for c in range(2)]
                acrow = [rows.tile([128, MW, P], f32, name=f"acrow{c}") for c in range(2)]
                ecrow = [rows.tile([128, MW, P], f32, name=f"ecrow{c}") for c in range(2)]
                for cc in range(2):
                    nc.sync.dma_start(xrow[cc][:], x_in[r, cc * 128:(cc + 1) * 128])
                    # batched centering of A over tokens within each patch
                    asum = rows.tile([128, MW], f32)
                    nc.vector.tensor_reduce(asum[:], xrow[cc][:], axis=AX.X,
                                            op=ALU.add)
                    nc.vector.scalar_tensor_tensor(
                        acrow[cc][:],
                        asum[:].unsqueeze(2).broadcast_to([128, MW, P]),
                        -1.0 / P,
                        xrow[cc][:],
                        ALU.mult,
                        ALU.add,
                    )

                for g in range(NG):
                    mw0 = g * G

                    def at(k, cc, row):
                        return row[cc][:, mw0 + k, :]

                    # MM1: S = A @ A^T per patch (contract over c, 2 chunks)
                    s_t = psS.tile([P, G, P], f32, name="pp")
                    for k in range(G):
                        nc.tensor.matmul(s_t[:, k, :], at(k, 0, xrow), at(k, 0, xrow),
                                         start=True, stop=False)
                        nc.tensor.matmul(s_t[:, k, :], at(k, 1, xrow), at(k, 1, xrow),
                                         start=False, stop=True)

                    # softmax over last dim, batched across G patches
                    negmx = grp.tile([P, G], f32)
                    nc.vector.tensor_reduce(negmx[:], s_t[:], axis=AX.X, op=ALU.max,
                                            negate=True)
                    ssub = grp.tile([P, G, P], f32)
                    nc.vector.scalar_tensor_tensor(
                        ssub[:], s_t[:], 0.0,
                        negmx[:].unsqueeze(2).broadcast_to([P, G, P]),
                        ALU.add, ALU.add)
                    e_sb = grp.tile([P, G, P], f32)
                    nc.scalar.activation(e_sb[:], ssub[:], ACT.Exp)
                    sums = grp.tile([P, G], f32)
                    nc.vector.tensor_reduce(sums[:], e_sb[:], axis=AX.X, op=ALU.add)
                    rec = grp.tile([P, G], f32)
                    nc.vector.reciprocal(rec[:], sums[:])
                    sc_sb = grp.tile([P, G, P], f32)
                    nc.vector.tensor_mul(sc_sb[:], e_sb[:],
                                         rec[:].unsqueeze(2).broadcast_to([P, G, P]))
                    nc.scalar.dma_start(
                        sc_o[r, mw0:mw0 + G].transpose([1, 0, 2]), sc_sb[:])

                    # ScT via PE transpose, one patch per slot (all base partition 0)
                    stp = psT.tile([P, G, P], f32, name="tp")
                    for k in range(G):
                        nc.tensor.transpose(stp[:, k, :], sc_sb[:, k, :],
                                            ident[0:P, 0:P])
                    sct_sb = grp.tile([P, G, P], f32)
                    nc.scalar.copy(sct_sb[:], stp[:])

                    def sct(k):
                        return sct_sb[:, k, :]

                    # A_p via PE transpose of raw A chunks, 4 slots per psum bank
                    ap_sb = grp.tile([P, 2 * G, 128], f32)
                    for t in range(G // 2):
                        app = psA.tile([P, 4, 128], f32, name="app")
                        for s in range(4):
                            k, cc = 2 * t + s // 2, s % 2
                            nc.tensor.transpose(app[:, s, :], at(k, cc, xrow),
                                                ident[:])
                        nc.scalar.copy(ap_sb[:, 4 * t:4 * t + 4, :], app[:])

                    def ap(k, cc):
                        return ap_sb[:, 2 * k + cc, :]

                    # MM2: McT = A_p^T @ ScT (per chunk)
                    mct_sb = []
                    msum = grp.tile([128, 2, G], f32)
                    for cc in range(2):
                        mct_t = psM.tile([128, G, P], f32, name="mm")
                        for k in range(G):
                            nc.tensor.matmul(mct_t[:, k, :], ap(k, cc), sct(k),
                                             start=True, stop=True)
                        msb = grp.tile([128, G, P], f32)
                        nc.scalar.copy(msb[:], mct_t[:])
                        nc.vector.tensor_reduce(msum[:, cc, :], msb[:], axis=AX.X,
                                                op=ALU.add)
                        mct_sb.append(msb)

                    # v = Ac @ colsum(Mc): rank-1 correction for mean-centering
                    v_t = psS.tile([P, G, P], f32, name="pp")
                    for k in range(G):
                        nc.tensor.matmul(v_t[:, k, 0:1], at(k, 0, acrow),
                                         msum[:, 0, k:k + 1], start=True, stop=False)
                        nc.tensor.matmul(v_t[:, k, 0:1], at(k, 1, acrow),
                                         msum[:, 1, k:k + 1], start=False, stop=True)
                    negv = grp.tile([P, G], f32)
                    nc.vector.tensor_scalar_mul(negv[:], v_t[:, :, 0], -1.0 / (P * P))

                    # MM3: cov_raw = Ac @ Mc^T (contract over c)
                    cov_t = psS.tile([P, G, P], f32, name="pp")
                    for k in range(G):
                        nc.tensor.matmul(cov_t[:, k, :], at(k, 0, acrow),
                                         mct_sb[0][:, k, :], start=True, stop=False)
                        nc.tensor.matmul(cov_t[:, k, :], at(k, 1, acrow),
                                         mct_sb[1][:, k, :], start=False, stop=True)
                    cov_sb = grp.tile([P, G, P], f32)
                    nc.vector.scalar_tensor_tensor(
                        cov_sb[:], cov_t[:], 1.0 / P,
                        negv[:].unsqueeze(2).broadcast_to([P, G, P]),
                        ALU.mult, ALU.add)
                    nc.scalar.dma_start(
                        cov_o[r, mw0:mw0 + G].transpose([1, 0, 2]), cov_sb[:])

                    l_sb = grp.tile([P, G, P], f32)
                    nc.vector.tensor_add(l_sb[:], sc_sb[:], cov_sb[:])
                    nc.scalar.dma_start(
                        l_o[r, mw0:mw0 + G].transpose([1, 0, 2]), l_sb[:])

                    # LT via PE transpose
                    ltp = psT.tile([P, G, P], f32, name="tp")
                    for k in range(G):
                        nc.tensor.transpose(ltp[:, k, :], l_sb[:, k, :],
                                            ident[0:P, 0:P])
                    lt_sb = grp.tile([P, G, P], f32)
                    nc.scalar.copy(lt_sb[:], ltp[:])

                    def lt(k):
                        return lt_sb[:, k, :]

                    # MM4: EcT = A_p^T @ LT, fold into ec row buffer
                    for cc in range(2):
                        ect_t = psM.tile([128, G, P], f32, name="mm")
                        for k in range(G):
                            nc.tensor.matmul(ect_t[:, k, :], ap(k, cc), lt(k),
                                             start=True, stop=True)
                        nc.scalar.copy(ecrow[cc][:, mw0:mw0 + G, :], ect_t[:])

                # row finalize: ec out, out = x*(beta*ec + x)
                for cc in range(2):
                    nc.sync.dma_start(ec_o[r, cc * 128:(cc + 1) * 128], ecrow[cc][:])
                    trow = rows.tile([128, MW, P], f32)
                    nc.vector.scalar_tensor_tensor(
                        trow[:], ecrow[cc][:], beta_sb[:, 0:1], xrow[cc][:],
                        ALU.mult, ALU.add)
                    orow = rows.tile([128, MW, P], f32)
                    nc.vector.tensor_mul(orow[:], trow[:], xrow[cc][:])
                    nc.sync.dma_start(out_o[r, cc * 128:(cc + 1) * 128], orow[:])

    nc.finalize()
    return nc


def _get_nc():
    global _cached_nc
    if _cached_nc is None:
        _cached_nc = _build_nc()
    return _cached_nc


def run(x, beta, trace=False):
    x = np.ascontiguousarray(np.asarray(x, dtype=np.float32))
    beta = np.asarray(beta, dtype=np.float32).reshape(1)
    # [B,C,H,W] -> [B,MH,C,MW,PH*PW] patch-major, sharded on fused (B,MH)
    xr = np.ascontiguousarray(
        x.reshape(B, C, MH, PH, MW, PW).transpose(0, 2, 1, 4, 3, 5)
    ).reshape(NCORES, RPC, C, MW, P)
    in_maps = [{"x": xr[i], "beta": beta} for i in range(NCORES)]
    res = run_bass_kernel_spmd(_get_nc(), in_maps, list(range(NCORES)), trace=trace)
    outs = res.results

    def grows(key):
        a = np.stack([outs[i][key] for i in range(NCORES)])
        return (a.reshape(B, MH, C, MW, PH, PW).transpose(0, 2, 1, 4, 3, 5)
                .reshape(B, C, H, W))

    def gpp(key):
        a = np.stack([outs[i][key] for i in range(NCORES)])
        return a.reshape(B, MH * MW, P, P)

    return (grows("out"), gpp("sc"), gpp("cov"), gpp("l"), grows("ec")), res


def kernel(x, beta, ph=7, pw=7):
    (out, sc, cov, l, ec), _ = run(x, beta, trace=False)
    return out, sc, cov, l, ec
